# revision 9
# baseline (speedup 1.0000x reference)
"""ECE loss kernel for Trainium2, data-parallel over 8 NeuronCores.

Strategy
--------
ECE = sum_b |sum_{i in bin b} (conf_i - acc_i)| / N, so the only binned
statistic needed per bin is d_b = sum(conf - acc). Per core (N/8 samples):

Host staging: softmaxes are downcast to fp16 (the 15-bin ECE statistic
tolerates far coarser conf quantization; measured rel err ~1e-5), and
plab[i] = sm16[i, label[i]] is gathered per sample. With plab staged,
accuracy needs no argmax on device: acc = (sm16[i,label]==max) differs
from first-argmax semantics only on exact fp16 ties (~2e-4 of samples).

Device per core, all on the Vector engine (DVE):
1. Stock tensor_reduce(max) over the 64-class axis of each fp16 tile
   [P, S, 64] -> conf16 [P, S]. All-fp16 SBUF operands make the stock op
   eligible for the DVE 2x/4x perf modes (a custom DVE op is locked to
   1 elem/cycle/lane), and fp16 halves the HBM traffic vs f32.
2. Per group: acc = is_equal(conf16, plab16); z = conf16 - acc (fp16).
3. 15 custom BIN_RANGE_SUM ops: accum_out = sum(z where lo < conf <= hi)
   per partition -> dstat slots.
4. Host: sum the per-core/per-partition stats in float64, abs, sum, /N.
"""

import sys

for _p in ("/opt/trn_rl_repo",):
    if _p not in sys.path:
        sys.path.insert(0, _p)

import numpy as np

import concourse.bass as bass
import concourse.mybir as mybir
import concourse.dve_spec as ds
import concourse.dve_ops as dops
from concourse.dve_spec import Spec, Src0, Src1, Zero, AluOp, lower, select
from concourse.dve_uop import DveOpSpec
from concourse.dve_ops import DveOp, OPS
from concourse.bass_utils import run_bass_kernel_spmd

# ----------------------------------------------------------------------------
# problem constants (hardcoded per the harness contract)
# ----------------------------------------------------------------------------
N_TOTAL = 4194304
C = 64
N_BINS = 15
CORES = 8
NC_SAMP = N_TOTAL // CORES        # 524288 samples per core
P = 128                           # SBUF partitions
S_TILE = 128                      # samples per partition per tile
TPG = 8                           # tiles per group
GROUPS = NC_SAMP // (P * S_TILE * TPG)   # 4
SG = S_TILE * TPG                 # samples per partition per group (1024)
SLOTS = GROUPS + TPG - 1 + 3      # dstat slot groups (drain splitting)

BOUNDS = np.linspace(0.0, 1.0, N_BINS + 1).astype(np.float32)


# ----------------------------------------------------------------------------
# custom DVE op: BIN_RANGE_SUM: out = (C0 < Src0 <= C1) ? Src1 : 0;
# accum_out = sum(out). fp16 inputs upconvert to f32 at the read ports.
# ----------------------------------------------------------------------------
def _make_op(name, spec_body, reference, subdim, accum=None):
    spec_kw = dict(body=spec_body, reference=reference)
    if accum is not None:
        spec_kw["accum"] = accum
    spec = Spec(**spec_kw)
    shas = {}
    for ver in ("v3", "v4"):
        uops = lower(spec, ver=ver)
        shas[ver] = DveOpSpec(
            name=name, opcode=0, uops=uops, rd1_en=ds._has_src1(spec)
        ).sha(ver)
    op = DveOp(name, spec, subdim=subdim, uops_sha=shas)
    if name not in dops._SUB_OPCODE_FOR_NAME:
        OPS.append(op)
        dops.CUSTOM_DVE_SPECS[name] = spec
        dops._SUB_OPCODE_FOR_NAME[name] = dops._CUSTOM_DVE_ROW_BASE + len(OPS) - 1
        assert dops._SUB_OPCODE_FOR_NAME[name] < 0x20
    else:
        op = next(o for o in OPS if o.name == name)
    return op


_inbin = ds.Bin(AluOp.LOGICAL_AND, Src0 > ds.C0, Src0 <= ds.C1)
_body2 = select(_inbin, Src1, Zero)


def _bin_range_sum_ref(in0, in1, s0, s1, imm2):
    x = np.asarray(in0, np.float32)
    z = np.asarray(in1, np.float32)
    out = np.where((x > s0) & (x <= s1), z, 0.0).astype(np.float32)
    acc = out.reshape(out.shape[0], -1).sum(axis=-1, keepdims=True).astype(np.float32)
    return out, acc


BIN_RANGE_SUM = _make_op(
    "BIN_RANGE_SUM_ANT", _body2, _bin_range_sum_ref, subdim=False, accum=AluOp.ADD
)

# ----------------------------------------------------------------------------
# bass program (one NEFF, run SPMD on 8 cores)
# ----------------------------------------------------------------------------
f32 = mybir.dt.float32
f16 = mybir.dt.float16

_NC_CACHE = {}

N_TILES = GROUPS * TPG
SM_BUFS = 6  # softmax tile ring depth
SPLIT_FIRST = True   # quarter-split the first softmax tile (startup latency)
SPLIT_LAST = True    # quarter-split the last softmax tile (drain latency)


def _build_nc(repeats: int = 1, variant: str = "full"):
    """Raw Bass (no Tile). repeats > 1 re-runs the identical workload
    back-to-back (for slope timing); every repeat recomputes the same dstat
    values, so results are unchanged. variant: "full" (normal), "dma"
    (loads only), "dve" (compute only) — roofline micro-benchmarks.
    """
    key = (repeats, variant)
    if key in _NC_CACHE:
        return _NC_CACHE[key]
    nc = bass.Bass()
    sm = nc.dram_tensor("sm", [NC_SAMP, C], f16, kind="ExternalInput")
    # plab pre-gathered+permuted on host to [partition, group*tile*sample]
    plab = nc.dram_tensor("plab", [P, GROUPS * SG], f16, kind="ExternalInput")
    thr = nc.dram_tensor("thr", [P, 16], f32, kind="ExternalInput")
    dstat = nc.dram_tensor("dstat", [P, SLOTS * 48], f32, kind="ExternalOutput")

    sm_v = sm.ap().rearrange(
        "(g t p s) c -> g t p (s c)", g=GROUPS, t=TPG, p=P, s=S_TILE
    )

    plab_sb = nc.alloc_sbuf_tensor("plab_sb", [P, GROUPS * SG], f16).ap()
    thr_sb = nc.alloc_sbuf_tensor("thr_sb", [P, 16], f32).ap()
    smt = [
        nc.alloc_sbuf_tensor(f"smt{i}", [P, S_TILE * C], f16).ap()
        for i in range(SM_BUFS)
    ]
    # pairwise max-tree scratch (sized for a full tile)
    ytree = [
        nc.alloc_sbuf_tensor(f"y{l}", [P, S_TILE * (C >> (l + 1))], f16).ap()
        for l in range(5)
    ]
    # conf is double-buffered per group: the Act engine reads group g's conf
    # while the DVE tree writes group g+1's.
    conf2 = [nc.alloc_sbuf_tensor(f"conf{i}", [P, SG], f16).ap() for i in range(2)]
    accf = nc.alloc_sbuf_tensor("accf", [P, SG], f16).ap()
    caccb = nc.alloc_sbuf_tensor("caccb", [P, SG], f16).ap()
    zbuf = nc.alloc_sbuf_tensor("zbuf", [P, SG], f16).ap()
    scrap_a = nc.alloc_sbuf_tensor("scrap_a", [P, SG], f16).ap()
    dstat_sb = nc.alloc_sbuf_tensor("dstat_sb", [P, SLOTS * 48], f32).ap()

    dsem = nc.alloc_semaphore()   # DMA-in completions (+16 each)
    vsem = nc.alloc_semaphore()   # DVE tile consumption (+1 per sm tile)
    asem = nc.alloc_semaphore()   # DVE conf production (+1 per unit L6)
    adone = nc.alloc_semaphore()  # Act slot-groups finished
    done = nc.alloc_semaphore()   # DVE fully done

    do_dma = variant in ("full", "dma")
    do_dve = variant in ("full", "dve")
    gate_on_dve = variant in ("full", "serial")
    serial = variant == "serial"
    if serial:
        do_dma = do_dve = True

    # Shared DMA/compute schedule: first and last softmax tiles are split
    # into quarters to shrink pipeline startup and drain.
    QS = S_TILE // 4  # samples per quarter
    units = []  # (tile_idx, quarter or None)
    split_tiles = {0: SPLIT_FIRST, N_TILES - 1: SPLIT_LAST}
    for i in range(N_TILES):
        if split_tiles.get(i):
            units.extend((i, q) for q in range(4))
        else:
            units.append((i, None))

    def unit_slices(i, q):
        """(g, t, smt_cols, conf_cols) for one unit."""
        g, t = divmod(i, TPG)
        if q is None:
            return (g, t, slice(0, S_TILE * C), slice(t * S_TILE, (t + 1) * S_TILE))
        return (
            g, t,
            slice(q * QS * C, (q + 1) * QS * C),
            slice(t * S_TILE + q * QS, t * S_TILE + (q + 1) * QS),
        )

    # ---- SP (sync) engine: all DMAs ----
    dcount = 0

    def dma(dst, srcv):
        nonlocal dcount
        nc.sync.dma_start(dst, srcv).then_inc(dsem, 16)
        dcount += 16
        return dcount

    unit_done = {}  # (r, unit_idx) -> dsem count when its DMA completed
    thr_count = dma(thr_sb[:], thr.ap()[:])
    if do_dma:
        for r in range(repeats):
            for ui, (i, q) in enumerate(units):
                g, t, smt_cols, _ = unit_slices(i, q)
                if q in (None, 0):
                    if serial and r > 0 and i == 0:
                        nc.sync.wait_ge(done, r)
                    ii = r * N_TILES + i
                    if gate_on_dve and ii >= SM_BUFS:
                        nc.sync.wait_ge(vsem, ii - SM_BUFS + 1)
                buf = smt[(r * N_TILES + i) % SM_BUFS]
                smv = sm_v[g, t]
                unit_done[(r, ui)] = dma(buf[:, smt_cols], smv[:, smt_cols])
                if r == 0 and i == 1:
                    dma(plab_sb[:], plab.ap()[:])
    else:
        dma(plab_sb[:], plab.ap()[:])
    if gate_on_dve:
        nc.sync.wait_ge(done, repeats)
        if do_dve:
            nc.sync.wait_ge(adone, repeats * SLOTS)
    dma(dstat.ap()[:], dstat_sb[:])
    nc.sync.wait_ge(dsem, dcount)

    U = len(units)
    T = [float(BOUNDS[b]) for b in range(N_BINS)]  # t_0=0 .. t_14=14/15

    # slot schedule shared by the DVE count streams and the Act relu
    # streams: (g, samples slice within group, slot idx, units prerequisite)
    slot_sched = []
    for g in range(GROUPS - 1):
        slot_sched.append((g, slice(0, SG), g, (g + 1) * TPG + 3))
    for t in range(TPG - 1):
        slot_sched.append(
            (GROUPS - 1, slice(t * S_TILE, (t + 1) * S_TILE),
             GROUPS - 1 + t, 4 + (GROUPS - 1) * TPG + t)
        )
    for qq in range(4):
        slot_sched.append(
            (GROUPS - 1,
             slice((TPG - 1) * S_TILE + qq * QS, (TPG - 1) * S_TILE + (qq + 1) * QS),
             GROUPS - 1 + TPG - 1 + qq, U - 4 + qq + 1)
        )
    assert len(slot_sched) == SLOTS

    # ---- DVE program ----
    def decode_and_count(g, sl, slot):
        """acc decode + per-threshold suffix counts into dstat slot group.
        Slot layout (48 f32): [0..13]=cnt_{b=1..14}, [14]=accsum,
        [16..29]=acccnt_{b=1..14}, [32..46]=relu_{b=0..14} (Act engine)."""
        n = sl.stop - sl.start
        cbuf = conf2[g % 2]
        nc.vector.tensor_tensor(
            out=accf[:, 0:n],
            in0=cbuf[:, sl],
            in1=plab_sb[:, g * SG + sl.start : g * SG + sl.stop],
            op=mybir.AluOpType.is_equal,
        )
        nc.vector.tensor_tensor(
            out=caccb[:, 0:n], in0=cbuf[:, sl], in1=accf[:, 0:n],
            op=mybir.AluOpType.mult,
        )
        nc.vector.tensor_scalar(
            out=zbuf[:, 0:n], in0=accf[:, 0:n], scalar1=0.0, scalar2=0.0,
            op0=mybir.AluOpType.add, op1=mybir.AluOpType.add,
            accum_out=dstat_sb[:, slot * 48 + 14 : slot * 48 + 15],
        )
        inst = None
        for b in range(1, N_BINS):
            inst = nc.vector.tensor_scalar(
                out=zbuf[:, 0:n], in0=cbuf[:, sl], scalar1=T[b], scalar2=0.0,
                op0=mybir.AluOpType.is_gt, op1=mybir.AluOpType.add,
                accum_out=dstat_sb[:, slot * 48 + b - 1 : slot * 48 + b],
            )
        for b in range(1, N_BINS):
            inst = nc.vector.tensor_scalar(
                out=zbuf[:, 0:n], in0=caccb[:, 0:n], scalar1=T[b], scalar2=0.0,
                op0=mybir.AluOpType.is_gt, op1=mybir.AluOpType.add,
                accum_out=dstat_sb[:, slot * 48 + 16 + b - 1 : slot * 48 + 16 + b],
            )
        return inst

    # adone value the DVE must see before overwriting conf2[g % 2] at
    # repeat r, group g (prior readers of that buffer must have finished).
    def conf_write_gate(r, g):
        if g == 0:
            return (r - 1) * SLOTS + 3 if r > 0 else 0
        if g == 1:
            return r * SLOTS if r > 0 else 0
        return r * SLOTS + (g - 1)

    for r in range(repeats if do_dve else 0):
        for ui, (i, q) in enumerate(units):
            g, t, smt_cols, conf_cols = unit_slices(i, q)
            if i % TPG == 0 and q in (None, 0):
                gate = conf_write_gate(r, g)
                if gate > 0:
                    nc.vector.wait_ge(adone, gate)
            if gate_on_dve:
                nc.vector.wait_ge(dsem, unit_done[(0 if not do_dma else r, ui)])
            buf = smt[(r * N_TILES + i) % SM_BUFS]
            n_samp = conf_cols.stop - conf_cols.start
            # 6-level pairwise max tree over contiguous class halves: every
            # level is a step-1 all-fp16 SBUF tensor_tensor -> DVE 2x_1p mode
            # (stock tensor_reduce only has a 1x uop program).
            src = buf[:, smt_cols].rearrange("p (s n) -> p s n", n=C)
            inst = None
            for l in range(6):
                w = C >> (l + 1)          # output classes per sample
                if l < 5:
                    dst = ytree[l][:, 0 : n_samp * w].rearrange(
                        "p (s n) -> p s n", n=w
                    )
                else:
                    dst = conf2[g % 2][:, conf_cols].rearrange(
                        "p (s n) -> p s n", n=1
                    )
                inst = nc.vector.tensor_tensor(
                    out=dst,
                    in0=src[:, :, 0:w],
                    in1=src[:, :, w : 2 * w],
                    op=mybir.AluOpType.max,
                )
                if l == 0 and (q is None or q == 3):
                    # tile buffer fully consumed after level 1
                    inst.then_inc(vsem, 1)
                src = dst
            inst.then_inc(asem, 1)  # conf for this unit is ready
            last_group = g == GROUPS - 1
            if last_group and i < N_TILES - 1 and q in (None, 3):
                decode_and_count(g, slice(t * S_TILE, (t + 1) * S_TILE),
                                 GROUPS - 1 + t)
            elif i == N_TILES - 1:
                if q is None:
                    inst = decode_and_count(
                        g, slice(t * S_TILE, (t + 1) * S_TILE), GROUPS - 1 + TPG - 1
                    )
                    inst.then_inc(done, 1)
                else:
                    inst = decode_and_count(
                        g,
                        slice(t * S_TILE + q * QS, t * S_TILE + (q + 1) * QS),
                        GROUPS - 1 + TPG - 1 + q,
                    )
                    if q == 3:
                        inst.then_inc(done, 1)
            elif (not last_group) and i % TPG == TPG - 1 and q in (None, 3):
                decode_and_count(g, slice(0, SG), g)

    # ---- Act (scalar) engine program: per-slot suffix relu sums ----
    if do_dve:
        nc.scalar.wait_ge(dsem, thr_count)
    for r in range(repeats if do_dve else 0):
        for g, sl, slot, need in slot_sched:
            n = sl.stop - sl.start
            nc.scalar.wait_ge(asem, r * U + need)
            inst = None
            for b in range(N_BINS):
                inst = nc.scalar.activation(
                    out=scrap_a[:, 0:n],
                    in_=conf2[g % 2][:, sl],
                    func=mybir.ActivationFunctionType.Relu,
                    bias=thr_sb[:, b : b + 1],
                    scale=1.0,
                    accum_out=dstat_sb[:, slot * 48 + 32 + b : slot * 48 + 33 + b],
                )
            inst.then_inc(adone, 1)

    # Raw Bass skips this pass; without it InstCustomDveAnt/.instr stays
    # empty and walrus fails with "ISA wrong length".
    mybir.codegen_inst_isa_subclasses(nc)
    _NC_CACHE[key] = nc
    return nc


# ----------------------------------------------------------------------------
# host staging shared by kernel() and the bench harness
# ----------------------------------------------------------------------------
def stage_inputs(softmaxes: np.ndarray, labels: np.ndarray):
    sm16 = np.asarray(softmaxes).astype(np.float16)
    assert sm16.shape == (N_TOTAL, C), sm16.shape
    lab = np.asarray(labels).astype(np.int64).ravel()
    plab = sm16[np.arange(N_TOTAL), lab]          # [N] fp16
    # permute to per-core [partition, group*tile*sample] layout
    plab = np.ascontiguousarray(
        plab.reshape(CORES, GROUPS, TPG, P, S_TILE).transpose(0, 3, 1, 2, 4)
    ).reshape(CORES, P, GROUPS * SG)
    thr_arr = np.broadcast_to(
        -np.linspace(0.0, 1.0, 16, dtype=np.float32)[None, :], (P, 16)
    ).copy()
    in_maps = []
    for k in range(CORES):
        in_maps.append(
            {
                "sm": np.ascontiguousarray(sm16[k * NC_SAMP : (k + 1) * NC_SAMP]),
                "plab": plab[k],
                "thr": thr_arr,
            }
        )
    return in_maps


# ----------------------------------------------------------------------------
# public entry point
# ----------------------------------------------------------------------------
def kernel(softmaxes: np.ndarray, labels: np.ndarray, _want_trace=False, _repeats=1):
    nc = _build_nc(_repeats)
    in_maps = stage_inputs(softmaxes, labels)
    res = run_bass_kernel_spmd(nc, in_maps, core_ids=list(range(CORES)))

    # aggregate per-threshold suffix stats (all linear -> sum across
    # cores/partitions/slots first, then apply the suffix algebra)
    CNT = np.zeros(N_BINS + 1, np.float64)    # cnt_b = #[conf > t_b], b=1..14
    ACC = np.zeros(N_BINS + 1, np.float64)    # acccnt_b, b=0..14
    RELU = np.zeros(N_BINS + 1, np.float64)   # relu_b = sum (conf-t_b)+, b=0..14
    for k in range(CORES):
        st = res.results[k]["dstat"].astype(np.float64)
        st = st.reshape(P, SLOTS, 48).sum(axis=(0, 1))
        CNT[1:N_BINS] += st[0:14]
        ACC[0] += st[14]
        ACC[1:N_BINS] += st[16:30]
        RELU[0:N_BINS] += st[32:47]

    T64 = np.linspace(0.0, 1.0, N_BINS + 1).astype(np.float32).astype(np.float64)
    A = np.zeros(N_BINS + 1, np.float64)      # A_b = sum conf over conf > t_b
    A[0:N_BINS] = RELU[0:N_BINS] + T64[0:N_BINS] * CNT[0:N_BINS]
    A[N_BINS] = 0.0                           # nothing above t=1
    ACC[N_BINS] = 0.0
    d = (A[:N_BINS] - A[1:]) - (ACC[:N_BINS] - ACC[1:])

    ece = np.float32(np.abs(d).sum() / N_TOTAL)
    out = np.array([ece], dtype=np.float32)
    if _want_trace:
        return out, res
    return out


# revision 11
# speedup vs baseline: 1.1738x; 1.1738x over previous
"""ECE loss kernel for Trainium2, data-parallel over 8 NeuronCores.

Strategy
--------
ECE = sum_b |sum_{i in bin b} (conf_i - acc_i)| / N, so the only binned
statistic needed per bin is d_b = sum(conf - acc). Per core (N/8 samples):

Host staging: softmaxes are downcast to fp16 (the 15-bin ECE statistic
tolerates far coarser conf quantization; measured rel err ~1e-5), and
plab[i] = sm16[i, label[i]] is gathered per sample. With plab staged,
accuracy needs no argmax on device: acc = (sm16[i,label]==max) differs
from first-argmax semantics only on exact fp16 ties (~2e-4 of samples).

Device per core, all on the Vector engine (DVE):
1. Per fp16 tile [P, S, 64]: a 6-level pairwise max tree over contiguous
   class halves (tensor_tensor(max, x[..., :w], x[..., w:2w])). Every
   level is a step-1 all-fp16 SBUF op, eligible for the DVE 2x_1p perf
   mode (2 elem/cycle/lane); stock tensor_reduce and custom DVE ops only
   have 1x uop programs. fp16 also halves the HBM traffic vs f32.
2. Per group: acc = is_equal(conf16, plab16); z = conf16 - acc (fp16).
3. 15 custom BIN_RANGE_SUM ops: accum_out = sum(z where lo < conf <= hi)
   per partition -> dstat slots.
4. Host: sum the per-core/per-partition stats in float64, abs, sum, /N.
"""

import sys

for _p in ("/opt/trn_rl_repo",):
    if _p not in sys.path:
        sys.path.insert(0, _p)

import numpy as np

import concourse.bass as bass
import concourse.mybir as mybir
import concourse.dve_spec as ds
import concourse.dve_ops as dops
from concourse.dve_spec import Spec, Src0, Src1, Zero, AluOp, lower, select
from concourse.dve_uop import DveOpSpec
from concourse.dve_ops import DveOp, OPS
from concourse.bass_utils import run_bass_kernel_spmd

# ----------------------------------------------------------------------------
# problem constants (hardcoded per the harness contract)
# ----------------------------------------------------------------------------
N_TOTAL = 4194304
C = 64
N_BINS = 15
CORES = 8
NC_SAMP = N_TOTAL // CORES        # 524288 samples per core
P = 128                           # SBUF partitions
S_TILE = 128                      # samples per partition per tile
TPG = 8                           # tiles per group
GROUPS = NC_SAMP // (P * S_TILE * TPG)   # 4
SG = S_TILE * TPG                 # samples per partition per group (1024)
SLOTS = GROUPS + TPG - 1 + 3      # dstat slot groups (drain splitting)

BOUNDS = np.linspace(0.0, 1.0, N_BINS + 1).astype(np.float32)


# ----------------------------------------------------------------------------
# custom DVE op: BIN_RANGE_SUM: out = (C0 < Src0 <= C1) ? Src1 : 0;
# accum_out = sum(out). fp16 inputs upconvert to f32 at the read ports.
# ----------------------------------------------------------------------------
def _make_op(name, spec_body, reference, subdim, accum=None):
    spec_kw = dict(body=spec_body, reference=reference)
    if accum is not None:
        spec_kw["accum"] = accum
    spec = Spec(**spec_kw)
    shas = {}
    for ver in ("v3", "v4"):
        uops = lower(spec, ver=ver)
        shas[ver] = DveOpSpec(
            name=name, opcode=0, uops=uops, rd1_en=ds._has_src1(spec)
        ).sha(ver)
    op = DveOp(name, spec, subdim=subdim, uops_sha=shas)
    if name not in dops._SUB_OPCODE_FOR_NAME:
        OPS.append(op)
        dops.CUSTOM_DVE_SPECS[name] = spec
        dops._SUB_OPCODE_FOR_NAME[name] = dops._CUSTOM_DVE_ROW_BASE + len(OPS) - 1
        assert dops._SUB_OPCODE_FOR_NAME[name] < 0x20
    else:
        op = next(o for o in OPS if o.name == name)
    return op


_inbin = ds.Bin(AluOp.LOGICAL_AND, Src0 > ds.C0, Src0 <= ds.C1)
_body2 = select(_inbin, Src1, Zero)


def _bin_range_sum_ref(in0, in1, s0, s1, imm2):
    x = np.asarray(in0, np.float32)
    z = np.asarray(in1, np.float32)
    out = np.where((x > s0) & (x <= s1), z, 0.0).astype(np.float32)
    acc = out.reshape(out.shape[0], -1).sum(axis=-1, keepdims=True).astype(np.float32)
    return out, acc


BIN_RANGE_SUM = _make_op(
    "BIN_RANGE_SUM_ANT", _body2, _bin_range_sum_ref, subdim=False, accum=AluOp.ADD
)

# ----------------------------------------------------------------------------
# bass program (one NEFF, run SPMD on 8 cores)
# ----------------------------------------------------------------------------
f32 = mybir.dt.float32
f16 = mybir.dt.float16

_NC_CACHE = {}

N_TILES = GROUPS * TPG
SM_BUFS = 6  # softmax tile ring depth
SPLIT_FIRST = True   # quarter-split the first softmax tile (startup latency)
SPLIT_LAST = True    # quarter-split the last softmax tile (drain latency)


def _build_nc(repeats: int = 1, variant: str = "full"):
    """Raw Bass (no Tile). repeats > 1 re-runs the identical workload
    back-to-back (for slope timing); every repeat recomputes the same dstat
    values, so results are unchanged. variant: "full" (normal), "dma"
    (loads only), "dve" (compute only) — roofline micro-benchmarks.
    """
    key = (repeats, variant)
    if key in _NC_CACHE:
        return _NC_CACHE[key]
    nc = bass.Bass()
    sm = nc.dram_tensor("sm", [NC_SAMP, C], f16, kind="ExternalInput")
    # plab pre-gathered+permuted on host to [partition, group*tile*sample]
    plab = nc.dram_tensor("plab", [P, GROUPS * SG], f16, kind="ExternalInput")
    dstat = nc.dram_tensor("dstat", [P, SLOTS * 16], f32, kind="ExternalOutput")

    sm_v = sm.ap().rearrange(
        "(g t p s) c -> g t p (s c)", g=GROUPS, t=TPG, p=P, s=S_TILE
    )

    plab_sb = nc.alloc_sbuf_tensor("plab_sb", [P, GROUPS * SG], f16).ap()
    smt = [
        nc.alloc_sbuf_tensor(f"smt{i}", [P, S_TILE * C], f16).ap()
        for i in range(SM_BUFS)
    ]
    # pairwise max-tree scratch (sized for a full tile)
    ytree = [
        nc.alloc_sbuf_tensor(f"y{l}", [P, S_TILE * (C >> (l + 1))], f16).ap()
        for l in range(5)
    ]
    conf = nc.alloc_sbuf_tensor("conf", [P, SG], f16).ap()
    accf = nc.alloc_sbuf_tensor("accf", [P, SG], f16).ap()
    zbuf = nc.alloc_sbuf_tensor("zbuf", [P, SG], f16).ap()
    dstat_sb = nc.alloc_sbuf_tensor("dstat_sb", [P, SLOTS * 16], f32).ap()
    scrap = nc.alloc_sbuf_tensor("scrap", [P, 1], f32).ap()

    dsem = nc.alloc_semaphore()   # DMA-in completions (+16 each)
    vsem = nc.alloc_semaphore()   # DVE tile consumption (+1 per sm tile)
    done = nc.alloc_semaphore()   # DVE fully done

    do_dma = variant in ("full", "dma")
    do_dve = variant in ("full", "dve")
    gate_on_dve = variant in ("full", "serial")
    serial = variant == "serial"
    if serial:
        do_dma = do_dve = True

    # Shared DMA/compute schedule: first and last softmax tiles are split
    # into quarters to shrink pipeline startup and drain.
    QS = S_TILE // 4  # samples per quarter
    units = []  # (tile_idx, quarter or None)
    split_tiles = {0: SPLIT_FIRST, N_TILES - 1: SPLIT_LAST}
    for i in range(N_TILES):
        if split_tiles.get(i):
            units.extend((i, q) for q in range(4))
        else:
            units.append((i, None))

    def unit_slices(i, q):
        """(g, t, smt_cols, conf_cols) for one unit."""
        g, t = divmod(i, TPG)
        if q is None:
            return (g, t, slice(0, S_TILE * C), slice(t * S_TILE, (t + 1) * S_TILE))
        return (
            g, t,
            slice(q * QS * C, (q + 1) * QS * C),
            slice(t * S_TILE + q * QS, t * S_TILE + (q + 1) * QS),
        )

    # ---- SP (sync) engine: all DMAs ----
    dcount = 0

    def dma(dst, srcv):
        nonlocal dcount
        nc.sync.dma_start(dst, srcv).then_inc(dsem, 16)
        dcount += 16
        return dcount

    unit_done = {}  # (r, unit_idx) -> dsem count when its DMA completed
    if do_dma:
        for r in range(repeats):
            for ui, (i, q) in enumerate(units):
                g, t, smt_cols, _ = unit_slices(i, q)
                if q in (None, 0):
                    if serial and r > 0 and i == 0:
                        nc.sync.wait_ge(done, r)
                    ii = r * N_TILES + i
                    if gate_on_dve and ii >= SM_BUFS:
                        nc.sync.wait_ge(vsem, ii - SM_BUFS + 1)
                buf = smt[(r * N_TILES + i) % SM_BUFS]
                smv = sm_v[g, t]
                unit_done[(r, ui)] = dma(buf[:, smt_cols], smv[:, smt_cols])
                if r == 0 and i == 1:
                    dma(plab_sb[:], plab.ap()[:])
    else:
        dma(plab_sb[:], plab.ap()[:])
    if gate_on_dve:
        nc.sync.wait_ge(done, repeats)
    dma(dstat.ap()[:], dstat_sb[:])
    nc.sync.wait_ge(dsem, dcount)

    # ---- DVE program ----
    def decode_and_bin(g, sl, slot):
        """acc/z decode + bin-reduce conf[:, sl] into dstat slot group."""
        n = sl.stop - sl.start
        nc.vector.tensor_tensor(
            out=accf[:, 0:n],
            in0=conf[:, sl],
            in1=plab_sb[:, g * SG + sl.start : g * SG + sl.stop],
            op=mybir.AluOpType.is_equal,
        )
        nc.vector.tensor_tensor(
            out=zbuf[:, 0:n], in0=conf[:, sl], in1=accf[:, 0:n],
            op=mybir.AluOpType.subtract,
        )
        inst = None
        for b in range(N_BINS):
            lo = float(BOUNDS[b])
            hi = 1.001 if b == N_BINS - 1 else float(BOUNDS[b + 1])
            inst = nc.vector._custom_dve(
                BIN_RANGE_SUM,
                out=scrap[:].broadcast_to([P, n]),
                accum_out=dstat_sb[:, slot * 16 + b : slot * 16 + b + 1],
                in0=conf[:, sl],
                in1=zbuf[:, 0:n],
                s0=lo,
                s1=hi,
            )
        return inst

    # dstat slot map: groups 0..G-2 group-level (slots 0..G-2); last group
    # per-tile (slots G-1 .. G+5), last tile per-quarter (slots G+6..G+9).
    for r in range(repeats if do_dve else 0):
        for ui, (i, q) in enumerate(units):
            g, t, smt_cols, conf_cols = unit_slices(i, q)
            if gate_on_dve:
                nc.vector.wait_ge(dsem, unit_done[(0 if not do_dma else r, ui)])
            buf = smt[(r * N_TILES + i) % SM_BUFS]
            n_samp = conf_cols.stop - conf_cols.start
            # 6-level pairwise max tree over contiguous class halves: every
            # level is a step-1 all-fp16 SBUF tensor_tensor -> DVE 2x_1p mode
            # (stock tensor_reduce only has a 1x uop program).
            src = buf[:, smt_cols].rearrange("p (s n) -> p s n", n=C)
            inst = None
            for l in range(6):
                w = C >> (l + 1)          # output classes per sample
                if l < 5:
                    dst = ytree[l][:, 0 : n_samp * w].rearrange(
                        "p (s n) -> p s n", n=w
                    )
                else:
                    dst = conf[:, conf_cols].rearrange("p (s n) -> p s n", n=1)
                inst = nc.vector.tensor_tensor(
                    out=dst,
                    in0=src[:, :, 0:w],
                    in1=src[:, :, w : 2 * w],
                    op=mybir.AluOpType.max,
                )
                if l == 0 and (q is None or q == 3):
                    # tile buffer fully consumed after level 1
                    inst.then_inc(vsem, 1)
                src = dst
            last_group = g == GROUPS - 1
            if last_group and i < N_TILES - 1 and q in (None, 3):
                decode_and_bin(g, slice(t * S_TILE, (t + 1) * S_TILE),
                               GROUPS - 1 + t)
            elif i == N_TILES - 1:
                if q is None:
                    inst = decode_and_bin(
                        g, slice(t * S_TILE, (t + 1) * S_TILE), GROUPS - 1 + TPG - 1
                    )
                    inst.then_inc(done, 1)
                else:
                    inst = decode_and_bin(
                        g,
                        slice(t * S_TILE + q * QS, t * S_TILE + (q + 1) * QS),
                        GROUPS - 1 + TPG - 1 + q,
                    )
                    if q == 3:
                        inst.then_inc(done, 1)
            elif (not last_group) and i % TPG == TPG - 1 and q in (None, 3):
                decode_and_bin(g, slice(0, SG), g)

    # Raw Bass skips this pass; without it InstCustomDveAnt/.instr stays
    # empty and walrus fails with "ISA wrong length".
    mybir.codegen_inst_isa_subclasses(nc)
    _NC_CACHE[key] = nc
    return nc


# ----------------------------------------------------------------------------
# host staging shared by kernel() and the bench harness
# ----------------------------------------------------------------------------
def stage_inputs(softmaxes: np.ndarray, labels: np.ndarray):
    sm16 = np.asarray(softmaxes).astype(np.float16)
    assert sm16.shape == (N_TOTAL, C), sm16.shape
    lab = np.asarray(labels).astype(np.int64).ravel()
    plab = sm16[np.arange(N_TOTAL), lab]          # [N] fp16
    # permute to per-core [partition, group*tile*sample] layout
    plab = np.ascontiguousarray(
        plab.reshape(CORES, GROUPS, TPG, P, S_TILE).transpose(0, 3, 1, 2, 4)
    ).reshape(CORES, P, GROUPS * SG)
    in_maps = []
    for k in range(CORES):
        in_maps.append(
            {
                "sm": np.ascontiguousarray(sm16[k * NC_SAMP : (k + 1) * NC_SAMP]),
                "plab": plab[k],
            }
        )
    return in_maps


# ----------------------------------------------------------------------------
# public entry point
# ----------------------------------------------------------------------------
def kernel(softmaxes: np.ndarray, labels: np.ndarray, _want_trace=False, _repeats=1):
    nc = _build_nc(_repeats)
    in_maps = stage_inputs(softmaxes, labels)
    res = run_bass_kernel_spmd(nc, in_maps, core_ids=list(range(CORES)))

    d = np.zeros(N_BINS, np.float64)
    for k in range(CORES):
        st = res.results[k]["dstat"].astype(np.float64)
        st = st.reshape(P, SLOTS, 16)[:, :, :N_BINS]
        d += st.sum(axis=(0, 1))

    ece = np.float32(np.abs(d).sum() / N_TOTAL)
    out = np.array([ece], dtype=np.float32)
    if _want_trace:
        return out, res
    return out


# revision 13
# speedup vs baseline: 2.5942x; 2.2101x over previous
"""ECE loss kernel for Trainium2, data-parallel over 8 NeuronCores.

Strategy
--------
ECE = sum_b |sum_{i in bin b} (conf_i - acc_i)| / N, so the only binned
statistic needed per bin is d_b = sum(conf - acc). Per core (N/8 samples):

Host staging: softmaxes are downcast to fp16 (the 15-bin ECE statistic
tolerates far coarser conf quantization; measured rel err ~1e-5), and
plab[i] = sm16[i, label[i]] is gathered per sample. With plab staged,
accuracy needs no argmax on device: acc = (sm16[i,label]==max) differs
from first-argmax semantics only on exact fp16 ties (~2e-4 of samples).

Device per core, all on the Vector engine (DVE):
1. Per fp16 tile [P, S, 64]: a 6-level pairwise max tree over contiguous
   class halves (tensor_tensor(max, x[..., :w], x[..., w:2w])). Every
   level is a step-1 all-fp16 SBUF op, eligible for the DVE 2x_1p perf
   mode (2 elem/cycle/lane); stock tensor_reduce and custom DVE ops only
   have 1x uop programs. fp16 also halves the HBM traffic vs f32.
2. Per group: acc = is_equal(conf16, plab16); z = conf16 - acc (fp16).
3. 15 custom BIN_RANGE_SUM ops: accum_out = sum(z where lo < conf <= hi)
   per partition -> dstat slots.
4. Host: sum the per-core/per-partition stats in float64, abs, sum, /N.
"""

import sys

for _p in ("/opt/trn_rl_repo",):
    if _p not in sys.path:
        sys.path.insert(0, _p)

import numpy as np

import concourse.bass as bass
import concourse.mybir as mybir
import concourse.dve_spec as ds
import concourse.dve_ops as dops
from concourse.dve_spec import Spec, Src0, Src1, Zero, AluOp, lower, select
from concourse.dve_uop import DveOpSpec
from concourse.dve_ops import DveOp, OPS
from concourse.bass_utils import run_bass_kernel_spmd

# ----------------------------------------------------------------------------
# problem constants (hardcoded per the harness contract)
# ----------------------------------------------------------------------------
N_TOTAL = 4194304
C = 64
N_BINS = 15
CORES = 8
NC_SAMP = N_TOTAL // CORES        # 524288 samples per core
P = 128                           # SBUF partitions
S_TILE = 128                      # samples per partition per tile
TPG = 8                           # tiles per group
GROUPS = NC_SAMP // (P * S_TILE * TPG)   # 4
SG = S_TILE * TPG                 # samples per partition per group (1024)
SLOTS = GROUPS + TPG - 1 + 3      # dstat slot groups (drain splitting)

BOUNDS = np.linspace(0.0, 1.0, N_BINS + 1).astype(np.float32)


# ----------------------------------------------------------------------------
# custom DVE op: BIN_RANGE_SUM: out = (C0 < Src0 <= C1) ? Src1 : 0;
# accum_out = sum(out). fp16 inputs upconvert to f32 at the read ports.
# ----------------------------------------------------------------------------
def _make_op(name, spec_body, reference, subdim, accum=None):
    spec_kw = dict(body=spec_body, reference=reference)
    if accum is not None:
        spec_kw["accum"] = accum
    spec = Spec(**spec_kw)
    shas = {}
    for ver in ("v3", "v4"):
        uops = lower(spec, ver=ver)
        shas[ver] = DveOpSpec(
            name=name, opcode=0, uops=uops, rd1_en=ds._has_src1(spec)
        ).sha(ver)
    op = DveOp(name, spec, subdim=subdim, uops_sha=shas)
    if name not in dops._SUB_OPCODE_FOR_NAME:
        OPS.append(op)
        dops.CUSTOM_DVE_SPECS[name] = spec
        dops._SUB_OPCODE_FOR_NAME[name] = dops._CUSTOM_DVE_ROW_BASE + len(OPS) - 1
        assert dops._SUB_OPCODE_FOR_NAME[name] < 0x20
    else:
        op = next(o for o in OPS if o.name == name)
    return op


_inbin = ds.Bin(AluOp.LOGICAL_AND, Src0 > ds.C0, Src0 <= ds.C1)
_body2 = select(_inbin, Src1, Zero)


def _bin_range_sum_ref(in0, in1, s0, s1, imm2):
    x = np.asarray(in0, np.float32)
    z = np.asarray(in1, np.float32)
    out = np.where((x > s0) & (x <= s1), z, 0.0).astype(np.float32)
    acc = out.reshape(out.shape[0], -1).sum(axis=-1, keepdims=True).astype(np.float32)
    return out, acc


BIN_RANGE_SUM = _make_op(
    "BIN_RANGE_SUM_ANT", _body2, _bin_range_sum_ref, subdim=False, accum=AluOp.ADD
)

# ----------------------------------------------------------------------------
# bass program (one NEFF, run SPMD on 8 cores)
# ----------------------------------------------------------------------------
f32 = mybir.dt.float32
f16 = mybir.dt.float16

_NC_CACHE = {}

N_TILES = GROUPS * TPG
SM_BUFS = 6  # softmax tile ring depth
SPLIT_FIRST = True   # quarter-split the first softmax tile (startup latency)
SPLIT_LAST = True    # quarter-split the last softmax tile (drain latency)


def _build_nc(repeats: int = 1, variant: str = "full"):
    """Raw Bass (no Tile). repeats > 1 re-runs the identical workload
    back-to-back (for slope timing); every repeat recomputes the same dstat
    values, so results are unchanged. variant: "full" (normal), "dma"
    (loads only), "dve" (compute only) — roofline micro-benchmarks.
    """
    key = (repeats, variant)
    if key in _NC_CACHE:
        return _NC_CACHE[key]
    nc = bass.Bass()
    sm = nc.dram_tensor("sm", [NC_SAMP, C], f16, kind="ExternalInput")
    # plab pre-gathered+permuted on host to [partition, group*tile*sample]
    plab = nc.dram_tensor("plab", [P, GROUPS * SG], f16, kind="ExternalInput")
    dstat = nc.dram_tensor("dstat", [P, SLOTS * 16], f32, kind="ExternalOutput")

    sm_v = sm.ap().rearrange(
        "(g t p s) c -> g t p (s c)", g=GROUPS, t=TPG, p=P, s=S_TILE
    )

    plab_sb = nc.alloc_sbuf_tensor("plab_sb", [P, GROUPS * SG], f16).ap()
    smt = [
        nc.alloc_sbuf_tensor(f"smt{i}", [P, S_TILE * C], f16).ap()
        for i in range(SM_BUFS)
    ]
    # pairwise max-tree scratch (sized for a full tile)
    ytree = [
        nc.alloc_sbuf_tensor(f"y{l}", [P, S_TILE * (C >> (l + 1))], f16).ap()
        for l in range(5)
    ]
    conf = nc.alloc_sbuf_tensor("conf", [P, SG], f16).ap()
    accf = nc.alloc_sbuf_tensor("accf", [P, SG], f16).ap()
    zbuf = nc.alloc_sbuf_tensor("zbuf", [P, SG], f16).ap()
    dstat_sb = nc.alloc_sbuf_tensor("dstat_sb", [P, SLOTS * 16], f32).ap()
    scrap = nc.alloc_sbuf_tensor("scrap", [P, 1], f32).ap()

    dsem = nc.alloc_semaphore()   # DMA-in completions (+16 each)
    vsem = nc.alloc_semaphore()   # DVE tile consumption (+1 per sm tile)
    done = nc.alloc_semaphore()   # DVE fully done

    do_dma = variant in ("full", "dma")
    do_dve = variant in ("full", "dve")
    gate_on_dve = variant in ("full", "serial")
    serial = variant == "serial"
    if serial:
        do_dma = do_dve = True

    # Shared DMA/compute schedule: first and last softmax tiles are split
    # into quarters to shrink pipeline startup and drain.
    QS = S_TILE // 4  # samples per quarter
    units = []  # (tile_idx, quarter or None)
    split_tiles = {0: SPLIT_FIRST, N_TILES - 1: SPLIT_LAST}
    for i in range(N_TILES):
        if split_tiles.get(i):
            units.extend((i, q) for q in range(4))
        else:
            units.append((i, None))

    def unit_slices(i, q):
        """(g, t, smt_cols, conf_cols) for one unit."""
        g, t = divmod(i, TPG)
        if q is None:
            return (g, t, slice(0, S_TILE * C), slice(t * S_TILE, (t + 1) * S_TILE))
        return (
            g, t,
            slice(q * QS * C, (q + 1) * QS * C),
            slice(t * S_TILE + q * QS, t * S_TILE + (q + 1) * QS),
        )

    # ---- SP (sync) engine: all DMAs ----
    dcount = 0

    def dma(dst, srcv):
        nonlocal dcount
        nc.sync.dma_start(dst, srcv).then_inc(dsem, 16)
        dcount += 16
        return dcount

    unit_done = {}  # (r, unit_idx) -> dsem count when its DMA completed
    if do_dma:
        for r in range(repeats):
            for ui, (i, q) in enumerate(units):
                g, t, smt_cols, _ = unit_slices(i, q)
                if q in (None, 0):
                    if serial and r > 0 and i == 0:
                        nc.sync.wait_ge(done, r)
                    ii = r * N_TILES + i
                    if gate_on_dve and ii >= SM_BUFS:
                        nc.sync.wait_ge(vsem, ii - SM_BUFS + 1)
                buf = smt[(r * N_TILES + i) % SM_BUFS]
                smv = sm_v[g, t]
                unit_done[(r, ui)] = dma(buf[:, smt_cols], smv[:, smt_cols])
                if r == 0 and i == 1:
                    dma(plab_sb[:], plab.ap()[:])
    else:
        dma(plab_sb[:], plab.ap()[:])
    if gate_on_dve:
        nc.sync.wait_ge(done, repeats)
    dma(dstat.ap()[:], dstat_sb[:])
    nc.sync.wait_ge(dsem, dcount)

    # ---- DVE program ----
    def decode_and_bin(g, sl, slot):
        """acc/z decode + bin-reduce conf[:, sl] into dstat slot group."""
        n = sl.stop - sl.start
        nc.vector.tensor_tensor(
            out=accf[:, 0:n],
            in0=conf[:, sl],
            in1=plab_sb[:, g * SG + sl.start : g * SG + sl.stop],
            op=mybir.AluOpType.is_equal,
        )
        nc.vector.tensor_tensor(
            out=zbuf[:, 0:n], in0=conf[:, sl], in1=accf[:, 0:n],
            op=mybir.AluOpType.subtract,
        )
        inst = None
        for b in range(N_BINS):
            lo = float(BOUNDS[b])
            hi = 1.001 if b == N_BINS - 1 else float(BOUNDS[b + 1])
            inst = nc.vector._custom_dve(
                BIN_RANGE_SUM,
                out=scrap[:].broadcast_to([P, n]),
                accum_out=dstat_sb[:, slot * 16 + b : slot * 16 + b + 1],
                in0=conf[:, sl],
                in1=zbuf[:, 0:n],
                s0=lo,
                s1=hi,
            )
        return inst

    # dstat slot map: groups 0..G-2 group-level (slots 0..G-2); last group
    # per-tile (slots G-1 .. G+5), last tile per-quarter (slots G+6..G+9).
    for r in range(repeats if do_dve else 0):
        for ui, (i, q) in enumerate(units):
            g, t, smt_cols, conf_cols = unit_slices(i, q)
            if gate_on_dve:
                nc.vector.wait_ge(dsem, unit_done[(0 if not do_dma else r, ui)])
            buf = smt[(r * N_TILES + i) % SM_BUFS]
            n_samp = conf_cols.stop - conf_cols.start
            # 6-level pairwise max tree over contiguous class halves: every
            # level is a step-1 all-fp16 SBUF tensor_tensor -> DVE 2x_1p mode
            # (stock tensor_reduce only has a 1x uop program).
            src = buf[:, smt_cols].rearrange("p (s n) -> p s n", n=C)
            inst = None
            for l in range(6):
                w = C >> (l + 1)          # output classes per sample
                if l < 5:
                    dst = ytree[l][:, 0 : n_samp * w].rearrange(
                        "p (s n) -> p s n", n=w
                    )
                else:
                    dst = conf[:, conf_cols].rearrange("p (s n) -> p s n", n=1)
                inst = nc.vector.tensor_tensor(
                    out=dst,
                    in0=src[:, :, 0:w],
                    in1=src[:, :, w : 2 * w],
                    op=mybir.AluOpType.max,
                )
                if l == 0 and (q is None or q == 3):
                    # tile buffer fully consumed after level 1
                    inst.then_inc(vsem, 1)
                src = dst
            last_group = g == GROUPS - 1
            if last_group and i < N_TILES - 1 and q in (None, 3):
                decode_and_bin(g, slice(t * S_TILE, (t + 1) * S_TILE),
                               GROUPS - 1 + t)
            elif i == N_TILES - 1:
                if q is None:
                    inst = decode_and_bin(
                        g, slice(t * S_TILE, (t + 1) * S_TILE), GROUPS - 1 + TPG - 1
                    )
                    inst.then_inc(done, 1)
                else:
                    inst = decode_and_bin(
                        g,
                        slice(t * S_TILE + q * QS, t * S_TILE + (q + 1) * QS),
                        GROUPS - 1 + TPG - 1 + q,
                    )
                    if q == 3:
                        inst.then_inc(done, 1)
            elif (not last_group) and i % TPG == TPG - 1 and q in (None, 3):
                decode_and_bin(g, slice(0, SG), g)

    # Raw Bass skips this pass; without it InstCustomDveAnt/.instr stays
    # empty and walrus fails with "ISA wrong length".
    mybir.codegen_inst_isa_subclasses(nc)
    _NC_CACHE[key] = nc
    return nc


# ----------------------------------------------------------------------------
# host staging shared by kernel() and the bench harness
# ----------------------------------------------------------------------------
def stage_inputs(softmaxes: np.ndarray, labels: np.ndarray):
    sm16 = np.asarray(softmaxes).astype(np.float16)
    assert sm16.shape == (N_TOTAL, C), sm16.shape
    lab = np.asarray(labels).astype(np.int64).ravel()
    plab = sm16[np.arange(N_TOTAL), lab]          # [N] fp16
    # permute to per-core [partition, group*tile*sample] layout
    plab = np.ascontiguousarray(
        plab.reshape(CORES, GROUPS, TPG, P, S_TILE).transpose(0, 3, 1, 2, 4)
    ).reshape(CORES, P, GROUPS * SG)
    in_maps = []
    for k in range(CORES):
        in_maps.append(
            {
                "sm": np.ascontiguousarray(sm16[k * NC_SAMP : (k + 1) * NC_SAMP]),
                "plab": plab[k],
            }
        )
    return in_maps


# ----------------------------------------------------------------------------
# public entry point
# ----------------------------------------------------------------------------
def kernel(softmaxes: np.ndarray, labels: np.ndarray, _want_trace=False, _repeats=1):
    nc = _build_nc(_repeats)
    in_maps = stage_inputs(softmaxes, labels)
    res = run_bass_kernel_spmd(nc, in_maps, core_ids=list(range(CORES)))

    d = np.zeros(N_BINS, np.float64)
    for k in range(CORES):
        st = res.results[k]["dstat"].astype(np.float64)
        st = st.reshape(P, SLOTS, 16)[:, :, :N_BINS]
        d += st.sum(axis=(0, 1))

    ece = np.float32(np.abs(d).sum() / N_TOTAL)
    out = np.array([ece], dtype=np.float32)
    if _want_trace:
        return out, res
    return out
